# revision 18
# baseline (speedup 1.0000x reference)
"""Trainium2 Bass kernel for nn_DeepSet_18356690223697 (moe_routing).

Strategy
--------
All per-edge linear algebra is folded, on the host, into functions of the
50-dim RBF expansion and two 100-row per-atom-type tables:

    d_t          = C * (edge_attr @ Wfold)          Wfold = Wd@Wdt
    expert_out_k = C*(edge_attr @ N_k) + P_k[z_src] + Q_k[z_dst]
    H            = C*(edge_attr @ Ng) + noise * softplus(C*(edge_attr @ Nn))
    gates        = sigmoid(+-(a - b)), a,b = top-2 of H   (expert slot k
                   always applies W_exp[k]; indices only order the gates)

(valid because every bias in setup_inputs() is zero - asserted below).

Sharding: atoms are padded to 20480 and split into 160 buckets of 128;
each of the 8 cores owns 20 buckets.  Edges are host-sorted by src bucket
and padded per bucket to a uniform tile count, so the per-atom segment-sum
becomes an on-chip one-hot scatter-matmul accumulating in PSUM and no
cross-core collective is needed (host concatenates per-core results).

Per 128-edge tile on device:
  attr  = exp(coeff*(o_r - w_e)^2) * C_e        [128e, 50]  (ACT+GPSIMD)
  attrT = PE transpose -> [50, 128]
  psum  = attrT.T @ W_all  (+ onehot(z_src).T @ P + onehot(z_dst).T @ Q)
  H-path -> g0/g1; payload = [edge_out | evn_d * edge_out (d=0,1,2)]
  psum_acc += onehot(src).T @ payload           (the segment sum)
"""
import sys

import numpy as np

sys.path.insert(0, "/opt/trn_rl_repo")

NATOMS = 20000
NEDGES = 400000
EMB = 256
NUM_RBF = 50
NGATES = 10
K = 2
FOUT = 128
OUTER = 5.0
NCORES = 8
NA_PAD = 20480          # 160 buckets * 128 atoms
NBUCK = NA_PAD // 128   # 160
BPC = NBUCK // NCORES   # 20 buckets per core

_OFFSET = np.linspace(0.0, OUTER, NUM_RBF).astype(np.float64)
_COEFF = -0.5 / (_OFFSET[1] - _OFFSET[0]) ** 2

_PROGRAM_CACHE = {}


def _fold_params(inp):
    f32 = np.float32
    for b in ["bd", "bdt", "bai", "baj", "bgam", "b_exp"]:
        assert not np.asarray(inp[b]).any(), f"nonzero bias {b} unsupported"
    Wd = np.asarray(inp["Wd"], np.float64)
    Wdt = np.asarray(inp["Wdt"], np.float64)
    Wgam = np.asarray(inp["Wgam"], np.float64)
    W_exp = np.asarray(inp["W_exp"], np.float64)
    W_g = np.asarray(inp["W_g"], np.float64)
    W_noise = np.asarray(inp["W_noise"], np.float64)
    emb = np.asarray(inp["emb"], np.float64)
    Wai = np.asarray(inp["Wai"], np.float64)
    Waj = np.asarray(inp["Waj"], np.float64)

    Wfold = Wd @ Wdt                                   # [50, 256]
    N = [Wfold @ (Wgam[0:EMB] @ W_exp[k]) for k in range(K)]      # [50,128]
    Ng = Wfold @ W_g                                   # [50, 10]
    Nn = Wfold @ W_noise                               # [50, 10]
    W_all = np.concatenate([N[0], N[1], Ng, Nn], axis=1).astype(f32)  # [50,276]
    Gi = (emb @ Wai) @ Wgam[EMB:2 * EMB]               # [100, 256]
    Gj = (emb @ Waj) @ Wgam[2 * EMB:3 * EMB]
    Ptab = np.concatenate([Gi @ W_exp[0], Gi @ W_exp[1]], axis=1).astype(f32)
    Qtab = np.concatenate([Gj @ W_exp[0], Gj @ W_exp[1]], axis=1).astype(f32)
    return W_all, Ptab, Qtab


def _bucketize(z, src, dst, w, ev, noise):
    """Sort edges by src bucket, pad each bucket to a uniform tile count."""
    E = src.shape[0]
    order = np.argsort(src, kind="stable")
    src_s = src[order]
    bucket = src_s >> 7
    counts = np.bincount(bucket, minlength=NBUCK)
    n_t = max(4, int(-(-counts.max() // 128)))
    cap = n_t * 128
    off = np.zeros(NBUCK + 1, np.int64)
    np.cumsum(counts, out=off[1:])
    pos = np.arange(E, dtype=np.int64) - off[bucket]
    slot = bucket.astype(np.int64) * cap + pos
    eid = np.full(NBUCK * cap, -1, np.int64)
    eid[slot] = order
    valid = eid >= 0
    e = np.where(valid, eid, 0)

    W = np.where(valid, w[e], 0.0).astype(np.float32)
    SR = np.where(valid, src[e].astype(np.int64) - (np.arange(NBUCK * cap) // cap) * 128,
                  -1).astype(np.float32)
    ZS = np.where(valid, z[src[e]], 0).astype(np.float32)
    ZD = np.where(valid, z[dst[e]], 0).astype(np.float32)
    EV = np.where(valid[:, None], ev[e], np.array([1.0, 0.0, 0.0], np.float32)
                  ).astype(np.float32)
    NO = np.where(valid[:, None], noise[e], 0.0).astype(np.float32)
    return n_t, W, SR, ZS, ZD, EV, NO


_CEN = 2.75


def _split3(x):
    """3-way bf16 split: x ~ h+m+l to ~24 mantissa bits."""
    import ml_dtypes
    bf = ml_dtypes.bfloat16
    h = x.astype(bf)
    r1 = x - h.astype(np.float64)
    m = r1.astype(bf)
    l = (r1 - m.astype(np.float64)).astype(bf)
    return h, m, l


def _build_core_inputs(c, n_t, W, SR, ZS, ZD, EV, NO):
    import ml_dtypes
    cap = n_t * 128
    TT = BPC * n_t
    QQ = TT // 4
    assert QQ <= 128, f"too many quads per core: {QQ}"
    lo, hi = c * BPC * cap, (c + 1) * BPC * cap
    srt = SR[lo:hi].reshape(TT, 128).T.copy()
    # wy9: per-quad rhs rows for the RBF-argument matmul
    #   psum[o, e] = o'*w' + y,  y = (coeff*w'^2 + ln C)/(-2*coeff)
    wq = W[lo:hi].astype(np.float64)
    wc = wq - _CEN
    Cq = 0.5 * (np.cos(wq * np.pi / OUTER) + 1.0)
    y = (_COEFF * wc * wc + np.log(Cq)) / (-2.0 * _COEFF)
    w1, w2, w3 = _split3(wc)
    y1, y2, y3 = _split3(y)
    rows = [w1, w2, w3, w1, w2, w1, y1, y2, y3]
    wy9 = np.zeros((QQ * 9, 512), ml_dtypes.bfloat16)
    for i, r in enumerate(rows):
        wy9[i::9] = r.reshape(QQ, 512)
    not_ = NO[lo:hi].reshape(TT, 128, NGATES).transpose(1, 0, 2).reshape(128, TT * NGATES).copy()
    evt = EV[lo:hi].reshape(TT, 128, 3).transpose(1, 0, 2).reshape(128, TT * 3).copy()
    import ml_dtypes
    zsq = np.zeros((128, 512), ml_dtypes.bfloat16)
    zsq[:QQ] = ZS[lo:hi].reshape(QQ, 512).astype(ml_dtypes.bfloat16)
    zdq = np.zeros((128, 512), ml_dtypes.bfloat16)
    zdq[:QQ] = ZD[lo:hi].reshape(QQ, 512).astype(ml_dtypes.bfloat16)
    return dict(wy9=wy9, srcrel_t=srt, noise_t=not_, evec_t=evt, zsq=zsq, zdq=zdq)


def _build_program(n_t):
    if n_t in _PROGRAM_CACHE:
        return _PROGRAM_CACHE[n_t]
    import concourse.bacc as bacc
    import concourse.bass as bass
    import concourse.mybir as mybir
    import concourse.tile as tile

    f32 = mybir.dt.float32
    f32r = mybir.dt.float32r
    bf16 = mybir.dt.bfloat16
    Alu = mybir.AluOpType
    Act = mybir.ActivationFunctionType

    TT = BPC * n_t

    nc = bacc.Bacc(None, target_bir_lowering=False)

    # DRAM I/O
    d_wall = nc.dram_tensor("W_all", [NUM_RBF, 276], f32, kind="ExternalInput")
    d_ptab = nc.dram_tensor("Ptab", [100, 256], f32, kind="ExternalInput")
    d_qtab = nc.dram_tensor("Qtab", [100, 256], f32, kind="ExternalInput")
    d_ow9 = nc.dram_tensor("ow9", [9, NUM_RBF], bf16, kind="ExternalInput")
    d_biaso = nc.dram_tensor("biaso", [NUM_RBF, 1], f32, kind="ExternalInput")
    d_iota128 = nc.dram_tensor("iota128", [128, 128], f32, kind="ExternalInput")
    d_iota100 = nc.dram_tensor("iota100", [100, 1], f32, kind="ExternalInput")
    d_ones100 = nc.dram_tensor("ones100", [1, 100], bf16, kind="ExternalInput")
    d_wy9 = nc.dram_tensor("wy9", [TT // 4 * 9, 512], bf16, kind="ExternalInput")
    d_srt = nc.dram_tensor("srcrel_t", [128, TT], f32, kind="ExternalInput")
    d_not = nc.dram_tensor("noise_t", [128, TT * NGATES], f32, kind="ExternalInput")
    d_evt = nc.dram_tensor("evec_t", [128, TT * 3], f32, kind="ExternalInput")
    d_zsq = nc.dram_tensor("zsq", [128, 512], bf16, kind="ExternalInput")
    d_zdq = nc.dram_tensor("zdq", [128, 512], bf16, kind="ExternalInput")
    d_out = nc.dram_tensor("out", [BPC * 128, 512], f32, kind="ExternalOutput")

    with tile.TileContext(nc) as tc:
        with (
            tc.tile_pool(name="res", bufs=1) as res,
            tc.tile_pool(name="quad", bufs=2) as quad,
            tc.tile_pool(name="work", bufs=3) as work,
            tc.tile_pool(name="pay", bufs=3) as pay,
            tc.tile_pool(name="pz", bufs=1, space="PSUM") as pz,
            tc.tile_pool(name="px", bufs=2, space="PSUM") as px,
            tc.tile_pool(name="pm", bufs=2, space="PSUM") as pm,
            tc.tile_pool(name="pacc", bufs=2, space="PSUM") as pacc,
        ):
            # ---- resident loads ----
            t_wall = res.tile([NUM_RBF, 276], f32)
            t_ptab = res.tile([100, 256], f32)
            t_qtab = res.tile([100, 256], f32)
            t_ow9 = res.tile([9, NUM_RBF], bf16)
            t_biaso = res.tile([NUM_RBF, 1], f32)
            t_iota128 = res.tile([128, 128], f32)
            t_iota100 = res.tile([100, 1], f32)
            t_ones100 = res.tile([1, 100], bf16)
            t_srt = res.tile([128, TT], f32)
            t_not = res.tile([128, TT * NGATES], f32)
            t_evt = res.tile([128, TT * 3], f32)
            # fp32r operand copies (matmul operands must be produced as fp32r)
            t_wallr = res.tile([NUM_RBF, 276], f32r)
            t_ptabr = res.tile([100, 256], f32r)
            t_qtabr = res.tile([100, 256], f32r)
            for t, d in [(t_wall, d_wall), (t_ptab, d_ptab), (t_qtab, d_qtab),
                         (t_ow9, d_ow9), (t_biaso, d_biaso),
                         (t_iota128, d_iota128),
                         (t_iota100, d_iota100), (t_ones100, d_ones100),
                         (t_srt, d_srt),
                         (t_not, d_not), (t_evt, d_evt)]:
                nc.sync.dma_start(t[:], d[:])
            nc.scalar.copy(t_wallr[:], t_wall[:])
            nc.scalar.copy(t_ptabr[:], t_ptab[:])
            nc.scalar.copy(t_qtabr[:], t_qtab[:])
            # ---- block-stage: normalized edge vectors for all tiles ----
            ev3 = t_evt[:].rearrange("p (t c) -> p t c", c=3)
            t_ev2 = res.tile([128, TT * 3], f32)
            nc.gpsimd.tensor_tensor(t_ev2[:], t_evt[:], t_evt[:], Alu.mult)
            e23 = t_ev2[:].rearrange("p (t c) -> p t c", c=3)
            t_n2 = res.tile([128, TT], f32)
            nc.vector.tensor_tensor(t_n2[:], e23[:, :, 0], e23[:, :, 1], Alu.add)
            t_n2b = res.tile([128, TT], f32)
            nc.vector.tensor_tensor(t_n2b[:], t_n2[:], e23[:, :, 2], Alu.add)
            t_nrm = res.tile([128, TT], f32)
            nc.scalar.activation(t_nrm[:], t_n2b[:], Act.Sqrt)
            t_rinv = res.tile([128, TT], f32)
            nc.vector.reciprocal(t_rinv[:], t_nrm[:])
            t_evn = res.tile([128, TT * 3], f32)
            evn3 = t_evn[:].rearrange("p (t c) -> p t c", c=3)
            for dd in range(3):
                nc.gpsimd.tensor_tensor(evn3[:, :, dd], ev3[:, :, dd],
                                        t_rinv[:], Alu.mult)

            ohS = None
            ohD = None
            for b in range(BPC):
                acc = pacc.tile([128, 512], f32, tag="acc")
                for t in range(n_t):
                    g = b * n_t + t
                    q, j = divmod(g, 4)
                    if j == 0:
                        # quad stage: one-hot(z) blocks for 4 tiles at once
                        # (matmul rhs must start at partition 0, so stage the
                        # quad's z rows through [1, 512] tiles via DMA)
                        zrs = quad.tile([1, 512], bf16, tag="zrs")
                        zrd = quad.tile([1, 512], bf16, tag="zrd")
                        nc.sync.dma_start(zrs[:], d_zsq[q:q + 1, :])
                        nc.sync.dma_start(zrd[:], d_zdq[q:q + 1, :])
                        pzt = pz.tile([100, 1024], f32, tag="pz")
                        nc.tensor.matmul(pzt[:, 0:512], t_ones100[:], zrs[:])
                        nc.tensor.matmul(pzt[:, 512:1024], t_ones100[:], zrd[:])
                        ohS = quad.tile([100, 512], f32r, tag="ohS")
                        ohD = quad.tile([100, 512], f32r, tag="ohD")
                        nc.vector.tensor_scalar(ohS[:], pzt[:, 0:512],
                                                t_iota100[:], None, Alu.is_equal)
                        nc.vector.tensor_scalar(ohD[:], pzt[:, 512:1024],
                                                t_iota100[:], None, Alu.is_equal)
                        # RBF argument for the quad:  psum = o'*w' + y
                        wy = quad.tile([9, 512], bf16, tag="wy")
                        nc.sync.dma_start(wy[:], d_wy9[q * 9:(q + 1) * 9, :])
                        pxt = px.tile([NUM_RBF, 512], f32, tag="px")
                        nc.tensor.matmul(pxt[:], t_ow9[:], wy[:])
                        attrq = quad.tile([NUM_RBF, 512], f32r, tag="attrq")
                        nc.scalar.activation(attrq[:], pxt[:], Act.Exp,
                                             scale=float(-2.0 * _COEFF),
                                             bias=t_biaso[:])

                    # ---- main matmuls -> psum_m [128, 276] ----
                    pmt = pm.tile([128, 512], f32, tag="pm")
                    nc.tensor.matmul(pmt[:, 0:276],
                                     attrq[:, j * 128:(j + 1) * 128],
                                     t_wallr[:],
                                     start=True, stop=False,
                                     skip_group_check=True)
                    nc.tensor.matmul(pmt[:, 0:256],
                                     ohS[:, j * 128:(j + 1) * 128],
                                     t_ptabr[:],
                                     start=False, stop=False,
                                     skip_group_check=True)
                    nc.tensor.matmul(pmt[:, 0:256],
                                     ohD[:, j * 128:(j + 1) * 128],
                                     t_qtabr[:],
                                     start=False, stop=True,
                                     skip_group_check=True)

                    # ---- H path -> gates ----
                    spe = work.tile([128, NGATES], f32, tag="spe")
                    nc.scalar.activation(spe[:], pmt[:, 266:276], Act.Exp)
                    sp = work.tile([128, NGATES], f32, tag="sp")
                    nc.scalar.activation(sp[:], spe[:], Act.Ln, bias=1.0)
                    nsp = work.tile([128, NGATES], f32, tag="nsp")
                    nc.gpsimd.tensor_tensor(
                        nsp[:], t_not[:, g * NGATES:(g + 1) * NGATES], sp[:],
                        Alu.mult)
                    H = work.tile([128, NGATES], f32, tag="H")
                    nc.vector.tensor_tensor(H[:], pmt[:, 256:266], nsp[:],
                                            Alu.add)
                    amax = work.tile([128, 1], f32, tag="amax")
                    nc.vector.tensor_reduce(amax[:], H[:],
                                            mybir.AxisListType.X, Alu.max)
                    pen = work.tile([128, NGATES], f32, tag="pen")
                    nc.gpsimd.tensor_scalar(pen[:], H[:], amax[:], -1e30,
                                            Alu.is_ge, Alu.mult)
                    hm = work.tile([128, NGATES], f32, tag="hm")
                    nc.gpsimd.tensor_tensor(hm[:], H[:], pen[:], Alu.add)
                    bmax = work.tile([128, 1], f32, tag="bmax")
                    nc.vector.tensor_reduce(bmax[:], hm[:],
                                            mybir.AxisListType.X, Alu.max)
                    dg = work.tile([128, 1], f32, tag="dg")
                    nc.gpsimd.tensor_tensor(dg[:], amax[:], bmax[:], Alu.subtract)
                    g0 = work.tile([128, 1], f32, tag="g0")
                    nc.scalar.activation(g0[:], dg[:], Act.Sigmoid)
                    g1 = work.tile([128, 1], f32, tag="g1")
                    nc.scalar.activation(g1[:], dg[:], Act.Sigmoid, scale=-1.0)

                    # ---- payload = [edge_out | evn_d * edge_out] ----
                    pl = pay.tile([128, 512], f32r, tag="pl")
                    tmp = work.tile([128, 128], f32, tag="tmp")
                    nc.scalar.activation(tmp[:], pmt[:, 128:256], Act.Copy,
                                         scale=g1[:])
                    nc.vector.scalar_tensor_tensor(
                        pl[:, 0:128], pmt[:, 0:128], g0[:], tmp[:],
                        Alu.mult, Alu.add)
                    nc.vector.tensor_scalar(pl[:, 128:256], pl[:, 0:128],
                                            t_evn[:, g * 3:g * 3 + 1], None,
                                            Alu.mult)
                    nc.gpsimd.tensor_scalar(pl[:, 256:384], pl[:, 0:128],
                                            t_evn[:, g * 3 + 1:g * 3 + 2], None,
                                            Alu.mult)
                    nc.gpsimd.tensor_scalar(pl[:, 384:512], pl[:, 0:128],
                                            t_evn[:, g * 3 + 2:g * 3 + 3], None,
                                            Alu.mult)

                    # ---- one-hot scatter-matmul (the segment sum) ----
                    oh = work.tile([128, 128], f32r, tag="oh")
                    nc.gpsimd.tensor_scalar(oh[:], t_iota128[:],
                                            t_srt[:, g:g + 1], None, Alu.is_equal)
                    nc.tensor.matmul(acc[:],
                                     oh[:],
                                     pl[:],
                                     start=(t == 0), stop=(t == n_t - 1),
                                     skip_group_check=True)

                acc_sb = pay.tile([128, 512], f32, tag="acc_sb")
                nc.scalar.copy(acc_sb[:], acc[:])
                nc.sync.dma_start(d_out[b * 128:(b + 1) * 128, :], acc_sb[:])

    nc.compile()
    _PROGRAM_CACHE[n_t] = nc
    return nc


def _consts():
    import ml_dtypes
    oc = _OFFSET - _CEN
    o1, o2, o3 = _split3(oc)
    ones = np.ones(NUM_RBF, ml_dtypes.bfloat16)
    ow9 = np.stack([o1, o1, o1, o2, o2, o3, ones, ones, ones]).astype(
        ml_dtypes.bfloat16)
    return dict(
        ow9=ow9,
        biaso=(_COEFF * oc * oc).astype(np.float32).reshape(NUM_RBF, 1),
        iota128=np.broadcast_to(np.arange(128, dtype=np.float32), (128, 128)).copy(),
        iota100=np.arange(100, dtype=np.float32).reshape(100, 1).copy(),
        ones100=np.ones((1, 100), ml_dtypes.bfloat16),
    )


def kernel(**inputs):
    from concourse.bass_utils import run_bass_kernel_spmd

    z = np.asarray(inputs["z"]).astype(np.int64)
    ei = np.asarray(inputs["edge_index"])
    src = ei[0].astype(np.int64)
    dst = ei[1].astype(np.int64)
    w = np.asarray(inputs["edge_weight"], np.float32)
    ev = np.asarray(inputs["edge_vec"], np.float32)
    noise = np.asarray(inputs["noise"], np.float32)

    W_all, Ptab, Qtab = _fold_params(inputs)
    n_t, W, SR, ZS, ZD, EV, NO = _bucketize(z, src, dst, w, ev, noise)
    nc = _build_program(n_t)

    consts = dict(W_all=W_all, Ptab=Ptab, Qtab=Qtab, **_consts())
    in_maps = []
    for c in range(NCORES):
        m = dict(consts)
        m.update(_build_core_inputs(c, n_t, W, SR, ZS, ZD, EV, NO))
        in_maps.append(m)

    res = run_bass_kernel_spmd(nc, in_maps, list(range(NCORES)))
    full = np.concatenate([res.results[c]["out"] for c in range(NCORES)], axis=0)
    atom_out = np.ascontiguousarray(full[:NATOMS, 0:FOUT])
    vec = np.ascontiguousarray(full[:NATOMS, FOUT:512].reshape(NATOMS, 3, FOUT))
    return atom_out, vec


# revision 19
# speedup vs baseline: 1.1309x; 1.1309x over previous
"""Trainium2 Bass kernel for nn_DeepSet_18356690223697 (moe_routing).

Strategy
--------
All per-edge linear algebra is folded, on the host, into functions of the
50-dim RBF expansion and two 100-row per-atom-type tables:

    d_t          = C * (edge_attr @ Wfold)          Wfold = Wd@Wdt
    expert_out_k = C*(edge_attr @ N_k) + P_k[z_src] + Q_k[z_dst]
    H            = C*(edge_attr @ Ng) + noise * softplus(C*(edge_attr @ Nn))
    gates        = sigmoid(+-(a - b)), a,b = top-2 of H   (expert slot k
                   always applies W_exp[k]; indices only order the gates)

(valid because every bias in setup_inputs() is zero - asserted below).

Sharding: atoms are padded to 20480 and split into 160 buckets of 128;
each of the 8 cores owns 20 buckets.  Edges are host-sorted by src bucket
and padded per bucket to a uniform tile count, so the per-atom segment-sum
becomes an on-chip one-hot scatter-matmul accumulating in PSUM and no
cross-core collective is needed (host concatenates per-core results).

Per 128-edge tile on device:
  attr  = exp(coeff*(o_r - w_e)^2) * C_e        [128e, 50]  (ACT+GPSIMD)
  attrT = PE transpose -> [50, 128]
  psum  = attrT.T @ W_all  (+ onehot(z_src).T @ P + onehot(z_dst).T @ Q)
  H-path -> g0/g1; payload = [edge_out | evn_d * edge_out (d=0,1,2)]
  psum_acc += onehot(src).T @ payload           (the segment sum)
"""
import sys

import numpy as np

sys.path.insert(0, "/opt/trn_rl_repo")

NATOMS = 20000
NEDGES = 400000
EMB = 256
NUM_RBF = 50
NGATES = 10
K = 2
FOUT = 128
OUTER = 5.0
NCORES = 8
NA_PAD = 20480          # 160 buckets * 128 atoms
NBUCK = NA_PAD // 128   # 160
BPC = NBUCK // NCORES   # 20 buckets per core

_OFFSET = np.linspace(0.0, OUTER, NUM_RBF).astype(np.float64)
_COEFF = -0.5 / (_OFFSET[1] - _OFFSET[0]) ** 2

_PROGRAM_CACHE = {}


def _fold_params(inp):
    f32 = np.float32
    for b in ["bd", "bdt", "bai", "baj", "bgam", "b_exp"]:
        assert not np.asarray(inp[b]).any(), f"nonzero bias {b} unsupported"
    Wd = np.asarray(inp["Wd"], np.float64)
    Wdt = np.asarray(inp["Wdt"], np.float64)
    Wgam = np.asarray(inp["Wgam"], np.float64)
    W_exp = np.asarray(inp["W_exp"], np.float64)
    W_g = np.asarray(inp["W_g"], np.float64)
    W_noise = np.asarray(inp["W_noise"], np.float64)
    emb = np.asarray(inp["emb"], np.float64)
    Wai = np.asarray(inp["Wai"], np.float64)
    Waj = np.asarray(inp["Waj"], np.float64)

    Wfold = Wd @ Wdt                                   # [50, 256]
    N = [Wfold @ (Wgam[0:EMB] @ W_exp[k]) for k in range(K)]      # [50,128]
    Ng = Wfold @ W_g                                   # [50, 10]
    Nn = Wfold @ W_noise                               # [50, 10]
    W_all = np.concatenate([N[0], N[1], Ng, Nn], axis=1).astype(f32)  # [50,276]
    Gi = (emb @ Wai) @ Wgam[EMB:2 * EMB]               # [100, 256]
    Gj = (emb @ Waj) @ Wgam[2 * EMB:3 * EMB]
    Ptab = np.concatenate([Gi @ W_exp[0], Gi @ W_exp[1]], axis=1).astype(f32)
    Qtab = np.concatenate([Gj @ W_exp[0], Gj @ W_exp[1]], axis=1).astype(f32)
    return W_all, Ptab, Qtab


def _bucketize(z, src, dst, w, ev, noise):
    """Sort edges by src bucket, pad each bucket to a uniform tile count."""
    E = src.shape[0]
    order = np.argsort(src, kind="stable")
    src_s = src[order]
    bucket = src_s >> 7
    counts = np.bincount(bucket, minlength=NBUCK)
    n_t = max(4, int(-(-counts.max() // 128)))
    cap = n_t * 128
    off = np.zeros(NBUCK + 1, np.int64)
    np.cumsum(counts, out=off[1:])
    pos = np.arange(E, dtype=np.int64) - off[bucket]
    slot = bucket.astype(np.int64) * cap + pos
    eid = np.full(NBUCK * cap, -1, np.int64)
    eid[slot] = order
    valid = eid >= 0
    e = np.where(valid, eid, 0)

    W = np.where(valid, w[e], 0.0).astype(np.float32)
    SR = np.where(valid, src[e].astype(np.int64) - (np.arange(NBUCK * cap) // cap) * 128,
                  -1).astype(np.float32)
    ZS = np.where(valid, z[src[e]], 0).astype(np.float32)
    ZD = np.where(valid, z[dst[e]], 0).astype(np.float32)
    EV = np.where(valid[:, None], ev[e], np.array([1.0, 0.0, 0.0], np.float32)
                  ).astype(np.float32)
    NO = np.where(valid[:, None], noise[e], 0.0).astype(np.float32)
    return n_t, W, SR, ZS, ZD, EV, NO


_CEN = 2.75


def _split3(x):
    """3-way bf16 split: x ~ h+m+l to ~24 mantissa bits."""
    import ml_dtypes
    bf = ml_dtypes.bfloat16
    h = x.astype(bf)
    r1 = x - h.astype(np.float64)
    m = r1.astype(bf)
    l = (r1 - m.astype(np.float64)).astype(bf)
    return h, m, l


def _build_core_inputs(c, n_t, W, SR, ZS, ZD, EV, NO):
    import ml_dtypes
    cap = n_t * 128
    TT = BPC * n_t
    QQ = TT // 4
    assert QQ <= 128, f"too many quads per core: {QQ}"
    lo, hi = c * BPC * cap, (c + 1) * BPC * cap
    srt = SR[lo:hi].reshape(TT, 128).T.copy()
    # wy9: per-quad rhs rows for the RBF-argument matmul
    #   psum[o, e] = o'*w' + y,  y = (coeff*w'^2 + ln C)/(-2*coeff)
    wq = W[lo:hi].astype(np.float64)
    wc = wq - _CEN
    Cq = 0.5 * (np.cos(wq * np.pi / OUTER) + 1.0)
    y = (_COEFF * wc * wc + np.log(Cq)) / (-2.0 * _COEFF)
    w1, w2, w3 = _split3(wc)
    y1, y2, y3 = _split3(y)
    rows = [w1, w2, w3, w1, w2, w1, y1, y2, y3]
    wy9 = np.zeros((QQ * 9, 512), ml_dtypes.bfloat16)
    for i, r in enumerate(rows):
        wy9[i::9] = r.reshape(QQ, 512)
    not_ = NO[lo:hi].reshape(TT, 128, NGATES).transpose(1, 0, 2).reshape(128, TT * NGATES).copy()
    evt = EV[lo:hi].reshape(TT, 128, 3).transpose(1, 0, 2).reshape(128, TT * 3).copy()
    import ml_dtypes
    zsq = np.zeros((128, 512), ml_dtypes.bfloat16)
    zsq[:QQ] = ZS[lo:hi].reshape(QQ, 512).astype(ml_dtypes.bfloat16)
    zdq = np.zeros((128, 512), ml_dtypes.bfloat16)
    zdq[:QQ] = ZD[lo:hi].reshape(QQ, 512).astype(ml_dtypes.bfloat16)
    return dict(wy9=wy9, srcrel_t=srt, noise_t=not_, evec_t=evt, zsq=zsq, zdq=zdq)


def _build_program(n_t):
    if n_t in _PROGRAM_CACHE:
        return _PROGRAM_CACHE[n_t]
    import concourse.bacc as bacc
    import concourse.bass as bass
    import concourse.mybir as mybir
    import concourse.tile as tile

    f32 = mybir.dt.float32
    f32r = mybir.dt.float32r
    bf16 = mybir.dt.bfloat16
    Alu = mybir.AluOpType
    Act = mybir.ActivationFunctionType

    TT = BPC * n_t

    nc = bacc.Bacc(None, target_bir_lowering=False)

    # DRAM I/O
    d_wall = nc.dram_tensor("W_all", [NUM_RBF, 276], f32, kind="ExternalInput")
    d_ptab = nc.dram_tensor("Ptab", [100, 256], f32, kind="ExternalInput")
    d_qtab = nc.dram_tensor("Qtab", [100, 256], f32, kind="ExternalInput")
    d_ow9 = nc.dram_tensor("ow9", [9, NUM_RBF], bf16, kind="ExternalInput")
    d_biaso = nc.dram_tensor("biaso", [NUM_RBF, 1], f32, kind="ExternalInput")
    d_iota128 = nc.dram_tensor("iota128", [128, 128], f32, kind="ExternalInput")
    d_iota100 = nc.dram_tensor("iota100", [100, 1], f32, kind="ExternalInput")
    d_ones100 = nc.dram_tensor("ones100", [1, 100], bf16, kind="ExternalInput")
    d_wy9 = nc.dram_tensor("wy9", [TT // 4 * 9, 512], bf16, kind="ExternalInput")
    d_srt = nc.dram_tensor("srcrel_t", [128, TT], f32, kind="ExternalInput")
    d_not = nc.dram_tensor("noise_t", [128, TT * NGATES], f32, kind="ExternalInput")
    d_evt = nc.dram_tensor("evec_t", [128, TT * 3], f32, kind="ExternalInput")
    d_zsq = nc.dram_tensor("zsq", [128, 512], bf16, kind="ExternalInput")
    d_zdq = nc.dram_tensor("zdq", [128, 512], bf16, kind="ExternalInput")
    d_out = nc.dram_tensor("out", [BPC * 128, 512], f32, kind="ExternalOutput")

    with tile.TileContext(nc) as tc:
        with (
            tc.tile_pool(name="res", bufs=1) as res,
            tc.tile_pool(name="quad", bufs=2) as quad,
            tc.tile_pool(name="work", bufs=3) as work,
            tc.tile_pool(name="pay", bufs=3) as pay,
            tc.tile_pool(name="pz", bufs=1, space="PSUM") as pz,
            tc.tile_pool(name="px", bufs=2, space="PSUM") as px,
            tc.tile_pool(name="pm", bufs=2, space="PSUM") as pm,
            tc.tile_pool(name="pacc", bufs=2, space="PSUM") as pacc,
        ):
            # ---- resident loads ----
            t_wall = res.tile([NUM_RBF, 276], f32)
            t_ptab = res.tile([100, 256], f32)
            t_qtab = res.tile([100, 256], f32)
            t_ow9 = res.tile([9, NUM_RBF], bf16)
            t_biaso = res.tile([NUM_RBF, 1], f32)
            t_iota128 = res.tile([128, 128], f32)
            t_iota100 = res.tile([100, 1], f32)
            t_ones100 = res.tile([1, 100], bf16)
            t_srt = res.tile([128, TT], f32)
            t_not = res.tile([128, TT * NGATES], f32)
            t_evt = res.tile([128, TT * 3], f32)
            # fp32r operand copies (matmul operands must be produced as fp32r)
            t_wallr = res.tile([NUM_RBF, 276], f32r)
            t_ptabr = res.tile([100, 256], f32r)
            t_qtabr = res.tile([100, 256], f32r)
            for t, d in [(t_wall, d_wall), (t_ptab, d_ptab), (t_qtab, d_qtab),
                         (t_ow9, d_ow9), (t_biaso, d_biaso),
                         (t_iota128, d_iota128),
                         (t_iota100, d_iota100), (t_ones100, d_ones100),
                         (t_srt, d_srt),
                         (t_not, d_not), (t_evt, d_evt)]:
                nc.sync.dma_start(t[:], d[:])
            nc.scalar.copy(t_wallr[:], t_wall[:])
            nc.scalar.copy(t_ptabr[:], t_ptab[:])
            nc.scalar.copy(t_qtabr[:], t_qtab[:])
            # ---- block-stage: normalized edge vectors for all tiles ----
            ev3 = t_evt[:].rearrange("p (t c) -> p t c", c=3)
            t_ev2 = res.tile([128, TT * 3], f32)
            nc.gpsimd.tensor_tensor(t_ev2[:], t_evt[:], t_evt[:], Alu.mult)
            e23 = t_ev2[:].rearrange("p (t c) -> p t c", c=3)
            t_n2 = res.tile([128, TT], f32)
            nc.vector.tensor_tensor(t_n2[:], e23[:, :, 0], e23[:, :, 1], Alu.add)
            t_n2b = res.tile([128, TT], f32)
            nc.vector.tensor_tensor(t_n2b[:], t_n2[:], e23[:, :, 2], Alu.add)
            t_nrm = res.tile([128, TT], f32)
            nc.scalar.activation(t_nrm[:], t_n2b[:], Act.Sqrt)
            t_rinv = res.tile([128, TT], f32)
            nc.vector.reciprocal(t_rinv[:], t_nrm[:])
            t_evn = res.tile([128, TT * 3], f32)
            evn3 = t_evn[:].rearrange("p (t c) -> p t c", c=3)
            for dd in range(3):
                nc.gpsimd.tensor_tensor(evn3[:, :, dd], ev3[:, :, dd],
                                        t_rinv[:], Alu.mult)

            ohS = None
            ohD = None
            for b in range(BPC):
                acc = pacc.tile([128, 512], f32, tag="acc")
                for t in range(n_t):
                    g = b * n_t + t
                    q, j = divmod(g, 4)
                    if j == 0:
                        # quad stage: one-hot(z) blocks for 4 tiles at once
                        # (matmul rhs must start at partition 0, so stage the
                        # quad's z rows through [1, 512] tiles via DMA)
                        zrs = quad.tile([1, 512], bf16, tag="zrs")
                        zrd = quad.tile([1, 512], bf16, tag="zrd")
                        nc.sync.dma_start(zrs[:], d_zsq[q:q + 1, :])
                        nc.sync.dma_start(zrd[:], d_zdq[q:q + 1, :])
                        pzt = pz.tile([100, 1024], f32, tag="pz")
                        nc.tensor.matmul(pzt[:, 0:512], t_ones100[:], zrs[:])
                        nc.tensor.matmul(pzt[:, 512:1024], t_ones100[:], zrd[:])
                        ohS = quad.tile([100, 512], f32r, tag="ohS")
                        ohD = quad.tile([100, 512], f32r, tag="ohD")
                        nc.vector.tensor_scalar(ohS[:], pzt[:, 0:512],
                                                t_iota100[:], None, Alu.is_equal)
                        nc.vector.tensor_scalar(ohD[:], pzt[:, 512:1024],
                                                t_iota100[:], None, Alu.is_equal)
                        # RBF argument for the quad:  psum = o'*w' + y
                        wy = quad.tile([9, 512], bf16, tag="wy")
                        nc.sync.dma_start(wy[:], d_wy9[q * 9:(q + 1) * 9, :])
                        pxt = px.tile([NUM_RBF, 512], f32, tag="px")
                        nc.tensor.matmul(pxt[:], t_ow9[:], wy[:])
                        attrq = quad.tile([NUM_RBF, 512], f32r, tag="attrq")
                        nc.scalar.activation(attrq[:], pxt[:], Act.Exp,
                                             scale=float(-2.0 * _COEFF),
                                             bias=t_biaso[:])

                    # ---- main matmuls -> psum_m [128, 276] ----
                    pmt = pm.tile([128, 512], f32, tag="pm")
                    nc.tensor.matmul(pmt[:, 0:276],
                                     attrq[:, j * 128:(j + 1) * 128],
                                     t_wallr[:],
                                     start=True, stop=False,
                                     skip_group_check=True)
                    nc.tensor.matmul(pmt[:, 0:256],
                                     ohS[:, j * 128:(j + 1) * 128],
                                     t_ptabr[:],
                                     start=False, stop=False,
                                     skip_group_check=True)
                    nc.tensor.matmul(pmt[:, 0:256],
                                     ohD[:, j * 128:(j + 1) * 128],
                                     t_qtabr[:],
                                     start=False, stop=True,
                                     skip_group_check=True)

                    # ---- H path -> gates ----
                    spe = work.tile([128, NGATES], f32, tag="spe")
                    nc.scalar.activation(spe[:], pmt[:, 266:276], Act.Exp)
                    sp = work.tile([128, NGATES], f32, tag="sp")
                    nc.scalar.activation(sp[:], spe[:], Act.Ln, bias=1.0)
                    nsp = work.tile([128, NGATES], f32, tag="nsp")
                    nc.gpsimd.tensor_tensor(
                        nsp[:], t_not[:, g * NGATES:(g + 1) * NGATES], sp[:],
                        Alu.mult)
                    H = work.tile([128, NGATES], f32, tag="H")
                    nc.vector.tensor_tensor(H[:], pmt[:, 256:266], nsp[:],
                                            Alu.add)
                    amax = work.tile([128, 1], f32, tag="amax")
                    nc.vector.tensor_reduce(amax[:], H[:],
                                            mybir.AxisListType.X, Alu.max)
                    pen = work.tile([128, NGATES], f32, tag="pen")
                    nc.gpsimd.tensor_scalar(pen[:], H[:], amax[:], -1e30,
                                            Alu.is_ge, Alu.mult)
                    hm = work.tile([128, NGATES], f32, tag="hm")
                    nc.gpsimd.tensor_tensor(hm[:], H[:], pen[:], Alu.add)
                    bmax = work.tile([128, 1], f32, tag="bmax")
                    nc.vector.tensor_reduce(bmax[:], hm[:],
                                            mybir.AxisListType.X, Alu.max)
                    dg = work.tile([128, 1], f32, tag="dg")
                    nc.gpsimd.tensor_tensor(dg[:], amax[:], bmax[:], Alu.subtract)
                    g0 = work.tile([128, 1], f32, tag="g0")
                    nc.scalar.activation(g0[:], dg[:], Act.Sigmoid)
                    g1 = work.tile([128, 1], f32, tag="g1")
                    nc.scalar.activation(g1[:], dg[:], Act.Sigmoid, scale=-1.0)

                    # ---- payload = [edge_out | evn_d * edge_out] ----
                    pl = pay.tile([128, 512], f32r, tag="pl")
                    tmp = work.tile([128, 128], f32, tag="tmp")
                    nc.scalar.activation(tmp[:], pmt[:, 128:256], Act.Copy,
                                         scale=g1[:])
                    nc.vector.scalar_tensor_tensor(
                        pl[:, 0:128], pmt[:, 0:128], g0[:], tmp[:],
                        Alu.mult, Alu.add)
                    nc.scalar.activation(pl[:, 128:256], pl[:, 0:128],
                                          Act.Copy,
                                          scale=t_evn[:, g * 3:g * 3 + 1])
                    nc.gpsimd.tensor_scalar(pl[:, 256:384], pl[:, 0:128],
                                            t_evn[:, g * 3 + 1:g * 3 + 2], None,
                                            Alu.mult)
                    nc.gpsimd.tensor_scalar(pl[:, 384:512], pl[:, 0:128],
                                            t_evn[:, g * 3 + 2:g * 3 + 3], None,
                                            Alu.mult)

                    # ---- one-hot scatter-matmul (the segment sum) ----
                    oh = work.tile([128, 128], f32r, tag="oh")
                    nc.gpsimd.tensor_scalar(oh[:], t_iota128[:],
                                            t_srt[:, g:g + 1], None, Alu.is_equal)
                    nc.tensor.matmul(acc[:],
                                     oh[:],
                                     pl[:],
                                     start=(t == 0), stop=(t == n_t - 1),
                                     skip_group_check=True)

                acc_sb = pay.tile([128, 512], f32, tag="acc_sb")
                nc.scalar.copy(acc_sb[:], acc[:])
                nc.sync.dma_start(d_out[b * 128:(b + 1) * 128, :], acc_sb[:])

    nc.compile()
    _PROGRAM_CACHE[n_t] = nc
    return nc


def _consts():
    import ml_dtypes
    oc = _OFFSET - _CEN
    o1, o2, o3 = _split3(oc)
    ones = np.ones(NUM_RBF, ml_dtypes.bfloat16)
    ow9 = np.stack([o1, o1, o1, o2, o2, o3, ones, ones, ones]).astype(
        ml_dtypes.bfloat16)
    return dict(
        ow9=ow9,
        biaso=(_COEFF * oc * oc).astype(np.float32).reshape(NUM_RBF, 1),
        iota128=np.broadcast_to(np.arange(128, dtype=np.float32), (128, 128)).copy(),
        iota100=np.arange(100, dtype=np.float32).reshape(100, 1).copy(),
        ones100=np.ones((1, 100), ml_dtypes.bfloat16),
    )


def kernel(**inputs):
    from concourse.bass_utils import run_bass_kernel_spmd

    z = np.asarray(inputs["z"]).astype(np.int64)
    ei = np.asarray(inputs["edge_index"])
    src = ei[0].astype(np.int64)
    dst = ei[1].astype(np.int64)
    w = np.asarray(inputs["edge_weight"], np.float32)
    ev = np.asarray(inputs["edge_vec"], np.float32)
    noise = np.asarray(inputs["noise"], np.float32)

    W_all, Ptab, Qtab = _fold_params(inputs)
    n_t, W, SR, ZS, ZD, EV, NO = _bucketize(z, src, dst, w, ev, noise)
    nc = _build_program(n_t)

    consts = dict(W_all=W_all, Ptab=Ptab, Qtab=Qtab, **_consts())
    in_maps = []
    for c in range(NCORES):
        m = dict(consts)
        m.update(_build_core_inputs(c, n_t, W, SR, ZS, ZD, EV, NO))
        in_maps.append(m)

    res = run_bass_kernel_spmd(nc, in_maps, list(range(NCORES)))
    full = np.concatenate([res.results[c]["out"] for c in range(NCORES)], axis=0)
    atom_out = np.ascontiguousarray(full[:NATOMS, 0:FOUT])
    vec = np.ascontiguousarray(full[:NATOMS, FOUT:512].reshape(NATOMS, 3, FOUT))
    return atom_out, vec
